# revision 1
# baseline (speedup 1.0000x reference)
"""CrossAttention Trainium2 kernel (v2).

Shapes (hardcoded from the problem spec):
  x  (32, 1024, 512) f32, xf (32, 77, 256) f32
  ln_g/ln_b (512,), tln_g/tln_b (256,)
  Wq (512,512), Wk (256,512), Wv (256,512), bq/bk/bv (512,)
  out y (32, 1024, 512) f32

Strategy (v2):
  - Data-parallel over batch: 32 batches -> 8 cores x 4 batches. No collectives.
  - Host folds LayerNorm gamma/beta and biases into the projection weights,
    casts to bf16, and ships BOTH x natural (for LN stats) and x^T
    (pre-transposed, for the Q projection). No on-device transposes of x.
  - LayerNorm of x never materializes: q = rstd*(x@Wq' - m*colsum(Wq')) + cq.
    The mean term is a K=1 matmul accumulated into the Q psum; rstd (and the
    1/sqrt(hd) scale) is applied by one tensor-multiply against a
    partition-broadcast rstd row during the Q PSUM->SBUF copy.
  - xf path: bn_stats layernorm -> PE transpose -> K^T and [V|1] projections.
  - S^T = k^T.T @ q^T per head (row-packed 2 heads per PE pass),
    P^T = exp(S^T) on ACT in 1024-wide calls, y = P^T.T @ [V|1] with the
    softmax denominator in column 64; normalize during PSUM->SBUF (gpsimd),
    y DMA'd out per half-batch for tail overlap.
"""

import numpy as np
import ml_dtypes

import concourse.bass as bass
import concourse.bacc as bacc
import concourse.mybir as mybir
import concourse.tile as tile
from concourse.bass_utils import run_bass_kernel_spmd
from concourse.masks import make_identity

B, T, D, N, L, H = 32, 1024, 512, 77, 256, 8
HD = D // H           # 64
NCORES = 8
BPC = B // NCORES     # 4 batches per core
EPS = 1e-5
SCALE = 1.0 / np.sqrt(HD)  # 0.125

BF16 = mybir.dt.bfloat16
F32 = mybir.dt.float32

DC = D // 128         # 4 D-chunks
LC = L // 128         # 2 L-chunks
XC = T // 128         # 8 T-chunks of natural x per batch


class _Bacc(bacc.Bacc):
    """Bacc whose ACT-table chooser only finds Exp/Ln in the combined
    natural_log_exp_and_others set, so the kernel needs one table load
    instead of ping-ponging between exp_and_others and the ln set."""

    def insert_act_table_loads(self):
        import bass_rust as _br
        from concourse.hw_specs import get_activation_tables

        has_activation = any(
            isinstance(i, mybir.InstActivation)
            for blk in self.main_func.blocks
            for i in blk.instructions
        )
        if not has_activation:
            return
        pair = {
            mybir.ActivationFunctionType.Exp,
            mybir.ActivationFunctionType.Ln,
        }
        tables = []
        for name, fns in get_activation_tables(self.m.arch).items():
            if name != "natural_log_exp_and_others":
                fns = fns - pair
            tables.append((name, fns))
        _br.insert_act_table_loads(self, tables)


def _build(bpc=BPC, has_cq=False, has_ck=False, has_cv=False):
    nc = _Bacc("TRN2", target_bir_lowering=False, debug=False)

    # All inputs pre-laid-out on host into SBUF tile shapes.
    xn_d = nc.dram_tensor("xn", (bpc, 128, XC, D), BF16, kind="ExternalInput")
    xt_d = nc.dram_tensor("xt", (bpc, 128, DC, T), BF16, kind="ExternalInput")
    xf_d = nc.dram_tensor("xf", (bpc, N, L), BF16, kind="ExternalInput")
    wq_d = nc.dram_tensor("wq", (128, DC, D), BF16, kind="ExternalInput")
    gq_d = nc.dram_tensor("gq", (1, DC, 128), BF16, kind="ExternalInput")
    wkv_d = nc.dram_tensor("wkv", (128, LC, 2 * D), BF16, kind="ExternalInput")
    cq_d = nc.dram_tensor("cq", (1, D), F32, kind="ExternalInput") if has_cq else None
    ck_d = nc.dram_tensor("ck", (1, D), F32, kind="ExternalInput") if has_ck else None
    cv_d = nc.dram_tensor("cv", (1, D), BF16, kind="ExternalInput") if has_cv else None
    y = nc.dram_tensor("y", (bpc, T, D), BF16, kind="ExternalOutput")

    with tile.TileContext(nc) as tc:
        _trace(tc, bpc, xn_d, xt_d, xf_d, wq_d, gq_d, wkv_d, cq_d, ck_d, cv_d, y)
    nc.compile()
    return nc


def _trace(tc, bpc, xn_d, xt_d, xf_d, wq_d, gq_d, wkv_d, cq_d, ck_d, cv_d, y):
    nc = tc.nc
    from contextlib import ExitStack

    ctx = ExitStack()
    with ctx:
        consts = ctx.enter_context(tc.tile_pool(name="consts", bufs=1))
        xfpool = ctx.enter_context(tc.tile_pool(name="xfpool", bufs=3))
        stats = ctx.enter_context(tc.tile_pool(name="stats", bufs=10))
        kvpool = ctx.enter_context(tc.tile_pool(name="kvpool", bufs=3))
        xpool = ctx.enter_context(tc.tile_pool(name="xpool", bufs=4))
        xtpool = ctx.enter_context(tc.tile_pool(name="xtpool", bufs=4))
        qpool = ctx.enter_context(tc.tile_pool(name="qpool", bufs=3))
        rowpool = ctx.enter_context(tc.tile_pool(name="rowpool", bufs=4))
        rbpool = ctx.enter_context(tc.tile_pool(name="rbpool", bufs=3))
        ptpool = ctx.enter_context(tc.tile_pool(name="ptpool", bufs=10))
        ypool = ctx.enter_context(tc.tile_pool(name="ypool", bufs=3))
        dramp = ctx.enter_context(tc.tile_pool(name="dramp", bufs=2, space="DRAM"))
        # PSUM: 8 banks: qps 2x1 + stp 2x1 + ypsp 2x2.
        qps = ctx.enter_context(tc.tile_pool(name="qps", bufs=2, space="PSUM"))
        stp = ctx.enter_context(tc.tile_pool(name="stp", bufs=2, space="PSUM"))
        ypsp = ctx.enter_context(tc.tile_pool(name="ypsp", bufs=2, space="PSUM"))

        # ---- constants ----
        wq_sb = consts.tile([128, DC, D], BF16, tag="wq")
        nc.scalar.dma_start(out=wq_sb, in_=wq_d[:])
        gq_sb = consts.tile([1, DC, 128], BF16, tag="gq")
        nc.scalar.dma_start(out=gq_sb, in_=gq_d[:])
        wkv_sb = consts.tile([128, LC, 2 * D], BF16, tag="wkv")
        nc.scalar.dma_start(out=wkv_sb, in_=wkv_d[:])
        eps_t = consts.tile([128, 1], F32, tag="eps")
        nc.vector.memset(eps_t, EPS)
        ident = consts.tile([128, 128], BF16, tag="ident")
        make_identity(nc, ident)
        cq_sb = ck_sb = cv_sb = None
        if cq_d is not None:
            cq_sb = consts.tile([128, DC], F32, tag="cq")  # [dout_part, chunk]
            nc.gpsimd.dma_start(
                out=cq_sb, in_=cq_d.rearrange("o (c p) -> (o p) c", p=128)
            )
        if ck_d is not None:
            ck_sb = consts.tile([128, DC], F32, tag="ck")
            nc.gpsimd.dma_start(
                out=ck_sb, in_=ck_d.rearrange("o (c p) -> (o p) c", p=128)
            )
        if cv_d is not None:
            cv_sb = consts.tile([1, D], BF16, tag="cv")
            nc.gpsimd.dma_start(out=cv_sb, in_=cv_d)
            ones_row = consts.tile([1, N], BF16, tag="ones_row")
            nc.vector.memset(ones_row, 1.0)

        kT_b, vt_b, xt_b, rb_b, mr_b = {}, {}, {}, {}, {}

        x_tiles = {}
        for b in range(bpc):
            x_t = xpool.tile([128, XC, D], BF16, tag="x")
            nc.sync.dma_start(out=x_t, in_=xn_d[b])
            xt_t = xtpool.tile([128, DC, T], BF16, tag="xt")
            nc.sync.dma_start(out=xt_t, in_=xt_d[b])
            x_tiles[b] = x_t
            xt_b[b] = xt_t

        def prep(b):
            x_t = x_tiles[b]
            xf_t = xfpool.tile([N, L], BF16, tag="xf")
            nc.gpsimd.dma_start(out=xf_t, in_=xf_d[b])

            # xf path: xfn, k^T, [V|1]
            st6 = stats.tile([N, 6], F32, tag="fst6")
            nc.vector.bn_stats(out=st6, in_=xf_t)
            mv_f = stats.tile([N, 2], F32, tag="fmv")
            nc.vector.bn_aggr(out=mv_f, in_=st6)
            rstd_f = stats.tile([N, 1], F32, tag="frstd")
            nc.scalar.activation(
                out=rstd_f, in_=mv_f[:, 1:2],
                func=mybir.ActivationFunctionType.Ln,
                bias=eps_t[:N], scale=1.0,
            )
            nc.scalar.activation(
                out=rstd_f, in_=rstd_f,
                func=mybir.ActivationFunctionType.Exp, scale=-0.5,
            )
            xfn = xfpool.tile([N, L], BF16, tag="xfn")
            nc.vector.tensor_scalar(
                out=xfn, in0=xf_t,
                scalar1=mv_f[:, 0:1], scalar2=rstd_f,
                op0=mybir.AluOpType.subtract, op1=mybir.AluOpType.mult,
            )
            xfnT = xfpool.tile([128, LC, N], BF16, tag="xfnT")
            for c in range(LC):
                tps = qps.tile([128, N], BF16, tag="fq")
                nc.tensor.transpose(
                    out=tps, in_=xfn[:, c * 128:(c + 1) * 128], identity=ident[:N, :N]
                )
                nc.scalar.copy(out=xfnT[:, c, :], in_=tps)

            kT = kvpool.tile([128, DC, N], BF16, tag="kT")
            for dc in range(DC):
                kps = qps.tile([128, N], F32, tag="fq")
                for lc in range(LC):
                    nc.tensor.matmul(
                        kps,
                        lhsT=wkv_sb[:, lc, dc * 128:(dc + 1) * 128],
                        rhs=xfnT[:, lc, :],
                        start=(lc == 0), stop=(lc == LC - 1),
                    )
                if ck_sb is not None:
                    nc.vector.tensor_scalar_add(
                        out=kps, in0=kps, scalar1=ck_sb[:, dc:dc + 1]
                    )
                nc.scalar.copy(out=kT[:, dc, :], in_=kps)
            kT_b[b] = kT

            vps = qps.tile([N, D], F32, tag="fq")
            for lc in range(LC):
                nc.tensor.matmul(
                    vps, lhsT=xfnT[:, lc, :], rhs=wkv_sb[:, lc, D:2 * D],
                    start=(lc == 0), stop=(lc == LC - 1 and cv_sb is None),
                )
            if cv_sb is not None:
                nc.tensor.matmul(vps, lhsT=ones_row, rhs=cv_sb, start=False, stop=True)
            vt = kvpool.tile([N, H, HD + 1], BF16, tag="vt")
            nc.scalar.copy(
                out=vt[:, :, 0:HD], in_=vps.rearrange("n (h d) -> n h d", h=H)
            )
            nc.vector.memset(vt[:, :, HD:HD + 1], 1.0)
            vt_b[b] = vt

            # x stats -> m/rstd rows -> broadcast
            mvx = stats.tile([128, XC, 2], F32, tag="mvx")
            for c in range(XC):
                s6 = stats.tile([128, 6], F32, tag="xst6")
                nc.vector.bn_stats(out=s6, in_=x_t[:, c, :])
                nc.vector.bn_aggr(out=mvx[:, c, :], in_=s6)
            srow = stats.tile([128, 2, XC], F32, tag="srow")
            nc.vector.tensor_copy(out=srow[:, 0, :], in_=mvx[:, :, 0])
            nc.scalar.activation(
                out=srow[:, 1, :], in_=mvx[:, :, 1:2],
                func=mybir.ActivationFunctionType.Ln,
                bias=eps_t, scale=1.0,
            )
            nc.scalar.activation(
                out=srow[:, 1, :], in_=srow[:, 1, :],
                func=mybir.ActivationFunctionType.Exp, scale=-0.5,
            )
            # gather to rows via DRAM bounce: row[0, x, c*128+p] = srow[p, x, c]
            scr = dramp.tile([2, XC, 128], F32, tag="scr")
            nc.gpsimd.dma_start(
                out=scr[:, :, :].rearrange("x c p -> p x c"), in_=srow
            )
            row2k = rowpool.tile([1, 2, T], F32, tag="row2k")
            nc.gpsimd.dma_start(
                out=row2k,
                in_=scr[:, :, :].rearrange("x c p -> () x (c p)"),
            )
            mrow_b = rowpool.tile([1, T], BF16, tag="mrow")
            nc.vector.tensor_copy(out=mrow_b, in_=row2k[:, 0, :])
            rrow_s = rowpool.tile([1, T], F32, tag="rrow")
            nc.vector.tensor_scalar_mul(
                out=rrow_s, in0=row2k[:, 1, :], scalar1=float(SCALE)
            )
            rstd_b = rbpool.tile([128, T], F32, tag="rstd_b")
            nc.gpsimd.partition_broadcast(rstd_b, rrow_s)
            mr_b[b] = mrow_b
            rb_b[b] = rstd_b

        def compute(b):
            xt_t, kT, vt = xt_b[b], kT_b[b], vt_b[b]
            mrow_b, rstd_b = mr_b[b], rb_b[b]
            for hf in range(2):
                ts = slice(hf * 512, (hf + 1) * 512)
                qt = qpool.tile([128, DC, 512], BF16, tag="qt")
                for dc in range(DC):
                    qp = qps.tile([128, 512], F32, tag="fq")
                    for kc in range(DC):
                        nc.tensor.matmul(
                            qp,
                            lhsT=wq_sb[:, kc, dc * 128:(dc + 1) * 128],
                            rhs=xt_t[:, kc, ts],
                            start=(kc == 0), stop=False,
                        )
                    # q -= m * colsum(Wq'): K=1 matmul, gq = -colsums
                    nc.tensor.matmul(
                        qp, lhsT=gq_sb[0:1, dc, :], rhs=mrow_b[0:1, ts],
                        start=False, stop=True,
                    )
                    # PSUM->SBUF copy fused with *(SCALE*rstd)
                    nc.vector.tensor_mul(
                        out=qt[:, dc, :], in0=qp, in1=rstd_b[:, ts]
                    )
                    if cq_sb is not None:
                        nc.vector.tensor_scalar_add(
                            out=qt[:, dc, :], in0=qt[:, dc, :],
                            scalar1=cq_sb[:, dc:dc + 1],
                        )

                pt_tiles = {}
                for hp in range(H // 2):
                    pt = ptpool.tile([N, 2, 512], BF16, tag="pt")
                    stqs = []
                    for sub in range(2):
                        po = 64 * sub
                        stq = stp.tile([N, 512], F32, tag="stq")
                        nc.tensor.matmul(
                            stq,
                            lhsT=kT[po:po + 64, hp, :],
                            rhs=qt[po:po + 64, hp, :],
                            start=True, stop=True,
                            tile_position=(po, 0),
                        )
                        stqs.append(stq)
                    for sub in range(2):
                        nc.scalar.activation(
                            out=pt[:, sub, :], in_=stqs[sub],
                            func=mybir.ActivationFunctionType.Exp,
                        )
                    pt_tiles[hp] = pt

                yh = ypool.tile([128, 4, D], BF16, tag="y")
                for ci in range(4):
                    # head h -> bank h//4, cols (h%4)*65..+65 (within-bank)
                    ypp = ypsp.tile([128, 2, 512], F32, tag="ypp")
                    ypv = ypp.rearrange("p j (h e) -> p j h e", h=4)
                    for h in range(H):
                        pt = pt_tiles[h // 2]
                        nc.tensor.matmul(
                            ypv[:, h // 4, h % 4, 0:HD + 1],
                            lhsT=pt[:, h % 2, ci * 128:(ci + 1) * 128],
                            rhs=vt[:, h, :],
                            start=True, stop=True,
                        )
                    rs = stats.tile([128, 2, 4], F32, tag="rs")
                    nc.vector.reciprocal(out=rs, in_=ypv[:, :, :, HD:HD + 1])
                    yv = yh[:, ci, :].rearrange("p (j h d) -> p j h d", j=2, h=4)
                    for j in range(2):
                        rs_ap = rs[:, j, :]
                        rs_bc = bass.AP(
                            tensor=rs_ap.tensor, offset=rs_ap.offset,
                            ap=[rs_ap.ap[0], rs_ap.ap[1], [0, HD]],
                        )
                        nc.vector.tensor_mul(
                            out=yv[:, j], in0=ypv[:, j, :, 0:HD], in1=rs_bc
                        )
                nc.gpsimd.dma_start(
                    out=y[b, hf * 512:(hf + 1) * 512].rearrange(
                        "(c p) d -> p c d", p=128
                    ),
                    in_=yh,
                )

        # software pipeline: prep runs ~2 batches ahead of compute
        for b in range(min(2, bpc)):
            prep(b)
        for b in range(bpc):
            compute(b)
            if b + 2 < bpc:
                prep(b + 2)


_CACHE = {}
TRACE = False          # set True to capture an NTFF profile on core 0
LAST_RESULTS = None    # BassKernelResults of the most recent kernel() call


def _get_nc(key):
    if key not in _CACHE:
        _CACHE[key] = _build(*key)
    return _CACHE[key]


def _lay_pcd(a, p=128):
    """[(c p), d] row-major -> [p, c, d] (partition-major SBUF layout)."""
    c = a.shape[0] // p
    return np.ascontiguousarray(a.reshape(c, p, a.shape[1]).transpose(1, 0, 2))


def kernel(x, xf, ln_g, ln_b, tln_g, tln_b, Wq, bq, Wk, bk, Wv, bv):
    x = np.asarray(x, np.float32)
    xf = np.asarray(xf, np.float32)
    bf = ml_dtypes.bfloat16
    # Fold layernorm affine + biases into the projections (f32 host math).
    wq_f = np.asarray(ln_g, np.float32)[:, None] * np.asarray(Wq, np.float32)
    cq = np.asarray(ln_b, np.float32) @ np.asarray(Wq, np.float32) + np.asarray(bq, np.float32)
    wk_f = np.asarray(tln_g, np.float32)[:, None] * np.asarray(Wk, np.float32)
    ck = np.asarray(tln_b, np.float32) @ np.asarray(Wk, np.float32) + np.asarray(bk, np.float32)
    wv_f = np.asarray(tln_g, np.float32)[:, None] * np.asarray(Wv, np.float32)
    cv = np.asarray(tln_b, np.float32) @ np.asarray(Wv, np.float32) + np.asarray(bv, np.float32)

    has_cq = bool(np.any(cq != 0))
    has_ck = bool(np.any(ck != 0))
    has_cv = bool(np.any(cv != 0))
    nc = _get_nc((BPC, has_cq, has_ck, has_cv))

    wq_b = wq_f.astype(bf)
    # mean-correction row: NEGATED column sums of the bf16 weight actually used
    gq = (-wq_b.astype(np.float32).sum(axis=0)).astype(bf)  # [512]
    wkv_b = np.concatenate([wk_f, wv_f], axis=1).astype(bf)  # [256, 1024]

    wq_lay = _lay_pcd(wq_b)                       # [128, 4, 512]
    gq_lay = np.ascontiguousarray(gq.reshape(1, DC, 128))
    wkv_lay = _lay_pcd(wkv_b)                     # [128, 2, 1024]

    x_b = x.astype(bf)
    xf_b = xf.astype(bf)

    in_maps = []
    for i in range(NCORES):
        xs = x_b[i * BPC:(i + 1) * BPC]
        xn_l = np.stack([_lay_pcd(xs[b]) for b in range(BPC)])            # [bpc,128,8,512]
        xt_l = np.stack([_lay_pcd(np.ascontiguousarray(xs[b].T)) for b in range(BPC)])  # [bpc,128,4,1024]
        m = {
            "xn": xn_l, "xt": xt_l,
            "xf": np.ascontiguousarray(xf_b[i * BPC:(i + 1) * BPC]),
            "wq": wq_lay, "gq": gq_lay, "wkv": wkv_lay,
        }
        if has_cq:
            # rstd_b carries SCALE, so the additive cq must carry it too
            m["cq"] = (cq * SCALE).reshape(1, D)
        if has_ck:
            m["ck"] = ck.reshape(1, D)
        if has_cv:
            m["cv"] = cv.reshape(1, D).astype(bf)
        in_maps.append(m)

    global LAST_RESULTS
    res = run_bass_kernel_spmd(
        nc, in_maps, core_ids=list(range(NCORES)), trace=TRACE
    )
    LAST_RESULTS = res
    out = np.concatenate([r["y"] for r in res.results], axis=0)
    return out.astype(np.float32)

